# revision 11
# baseline (speedup 1.0000x reference)
"""Multi-head attention (B=8, N=1024, D=768, H=12) on 8 TRN2 NeuronCores.

Data-parallel: one batch element per core, no collectives. Per-core kernel:
  xT = x.T                          (PE transpose, bf16)
  qkT[c, t] = w_qkv[:, c].T @ xT    (c in [0, 1536): q rows then k rows,
                                     bias fused, q pre-scaled by 1/8)
  v[t, c]   = xT.T @ w_qkv[:, 1536:]  (natural layout; bias folded later --
                                       softmax rows sum to 1, so
                                       attn @ (v + b) = attn @ v + b)
  per head: scoresT[k, q] = kT.T @ qT ; expT = exp(scoresT)  (no max
            subtraction -- scores are O(5) for randn inputs, exp is safe)
            aT+sums = vpack.T @ expT  (ones column packed into vpack emits
            softmax denominators in the same matmul)
            aT = aT * (1/sums) + b_v  (partition-broadcast of 1/sums)
  out = aT.T @ w_proj + b_proj

All matmul operands bf16 (PSUM accumulation fp32, softmax fp32).
"""

import os

import numpy as np

import bass_rust
from bass_rust import ScopedClock

import concourse.bass as bass
import concourse.tile as tile
from concourse import mybir
from concourse.bass_utils import run_bass_kernel_spmd
from concourse.masks import make_identity

# ---------------------------------------------------------------------------
# Workarounds: this container's walrus allows only ONE sync wait per
# instruction ("Too many sync wait commands"). Split extras onto same-engine
# NoOps (engine sequencers execute in program order).
# ---------------------------------------------------------------------------
_MAX_WAITS = 1


def _patched_drain_and_barrier(self, tick_clock, wait_clock):
    nc = self.nc
    drain_inst = nc.sync.drain()
    wait_clock.add_sem_waits(
        drain_inst.ins, ScopedClock({None: tick_clock.global_clock})
    )
    waits = list(drain_inst.ins.sync_info.on_wait)
    if len(waits) > _MAX_WAITS:
        drain_inst.ins.sync_info = bass_rust.SyncInfo(on_wait=[], on_update=[])
        by_num = {h.num: h for h in self.sems.allocated().values()}
        for w in waits:
            h = by_num.get(w.id)
            if h is None:
                h = bass_rust.SemaphoreHandle(name=w.ant_name, num=w.id)
            nc.sync.wait_ge(h, w.wait_value)

    nc.all_engine_barrier()
    assert self.sems is not None
    popped = nc._tile_sem_poison_stack.pop()
    assert popped is self._sem_poison
    nc.clear_and_free_semaphores(list(self.sems.allocated().values()))
    nc.all_engine_barrier()


tile.TileContext._drain_and_barrier = _patched_drain_and_barrier


def _legalize_waits(nc):
    n_split = 0
    for fn in nc.m.functions:
        for bb in fn.blocks:
            insts = bb.instructions
            if not any(
                i.sync_info is not None and len(i.sync_info.on_wait) > _MAX_WAITS
                for i in insts
            ):
                continue
            new = []
            for inst in insts:
                si = inst.sync_info
                if si is not None and len(si.on_wait) > _MAX_WAITS:
                    waits = list(si.on_wait)
                    keep, extra = waits[:_MAX_WAITS], waits[_MAX_WAITS:]
                    for j, w in enumerate(extra):
                        nop = mybir.InstNoOp(
                            name=f"{inst.name}-ws{j}", ins=[], outs=[],
                            engine=inst.engine,
                        )
                        nop.sync_info = bass_rust.SyncInfo(on_wait=[w], on_update=[])
                        new.append(nop)
                        n_split += 1
                    inst.sync_info = bass_rust.SyncInfo(
                        on_wait=keep, on_update=list(si.on_update)
                    )
                new.append(inst)
            bb.instructions = new
    return n_split


# ---------------------------------------------------------------------------
# Kernel builder (per-core shapes hardcoded: x [1024, 768])
# ---------------------------------------------------------------------------
N, D, H, HD = 1024, 768, 12, 64
NT = N // 128       # 8 token chunks
DC = D // 128       # 6 d chunks
CT = (2 * D) // 128  # 12 qk col tiles
KC = N // 128       # 8 key chunks
SCALE = HD ** -0.5

F32 = mybir.dt.float32
BF16 = mybir.dt.bfloat16
Exp = mybir.ActivationFunctionType.Exp
ADD = mybir.AluOpType.add
MULT = mybir.AluOpType.mult


def build(legalize=True):
    nc = bass.Bass()
    x_d = nc.declare_dram_parameter("x", [N, D], F32, isOutput=False)
    wqkv_d = nc.declare_dram_parameter("w_qkv", [D, 3 * D], F32, isOutput=False)
    bqkv_d = nc.declare_dram_parameter("b_qkv", [3 * D], F32, isOutput=False)
    wp_d = nc.declare_dram_parameter("w_proj", [D, D], F32, isOutput=False)
    bp_d = nc.declare_dram_parameter("b_proj", [D], F32, isOutput=False)
    out_d = nc.declare_dram_parameter("out", [N, D], F32, isOutput=True)

    with tile.TileContext(nc) as tc:
        with (
            tc.tile_pool(name="persist", bufs=1) as persist,
            tc.tile_pool(name="consts", bufs=1) as consts,
        ):
            qkT = persist.tile([128, CT, N], BF16)         # [qk col, tok]
            vpack = persist.tile([128, KC, 6, 2, 128], BF16)
            aT = persist.tile([128, DC, N], BF16)          # [d, tok]
            wp_sb = persist.tile([128, DC, D], BF16)
            bqk_sb = consts.tile([128, CT], F32)
            bv_sb = consts.tile([128, DC], F32)
            bproj_bc = consts.tile([128, D], F32)
            ident = consts.tile([128, 128], BF16)

            make_identity(nc, ident[:])
            nc.sync.dma_start(
                bqk_sb[:], bqkv_d.ap()[: 2 * D].rearrange("(o i) -> i o", i=128)
            )
            nc.sync.dma_start(
                bv_sb[:], bqkv_d.ap()[2 * D :].rearrange("(o i) -> i o", i=128)
            )
            bp_ap = bp_d.ap()
            nc.sync.dma_start(
                bproj_bc[:],
                bass.AP(tensor=bp_ap.tensor, offset=bp_ap.offset,
                        ap=[[0, 128]] + bp_ap.ap),
            )
            # ones columns of vpack (even head: col 64; odd head: col 0)
            nc.vector.memset(vpack[:], 0.0)
            nc.vector.memset(vpack[:, :, :, 0, 64:65], 1.0)
            nc.vector.memset(vpack[:, :, :, 1, 0:1], 1.0)

            # ---------------- phase 1: xT, qkT, v ----------------
            with (
                tc.tile_pool(name="p1", bufs=2) as p1,
                tc.tile_pool(name="p1big", bufs=1) as p1big,
                tc.tile_pool(name="ps_tp", bufs=2, space="PSUM") as ps_tp,
                tc.tile_pool(name="ps_qk", bufs=2, space="PSUM") as ps_qk,
                tc.tile_pool(name="ps_v", bufs=2, space="PSUM") as ps_v,
            ):
                xT = p1big.tile([128, DC, NT, 128], BF16)  # [d, tok]
                wqk = p1big.tile([128, DC, 3 * D], BF16)

                for kc in range(DC):
                    wst = p1.tile([128, 3 * D], F32, tag="wst")
                    nc.sync.dma_start(
                        wst[:], wqkv_d.ap()[kc * 128 : (kc + 1) * 128, :]
                    )
                    nc.any.tensor_copy(wqk[:, kc, :], wst[:])

                for t in range(NT):
                    xst = p1.tile([128, D], F32, tag="xst")
                    nc.sync.dma_start(
                        xst[:], x_d.ap()[t * 128 : (t + 1) * 128, :]
                    )
                    xbf = p1.tile([128, D], BF16, tag="xbf")
                    nc.any.tensor_copy(xbf[:], xst[:])
                    for dc in range(DC):
                        tp = ps_tp.tile([128, 128], BF16, tag="tp")
                        nc.tensor.transpose(
                            tp[:], xbf[:, dc * 128 : (dc + 1) * 128], ident[:]
                        )
                        nc.any.tensor_copy(xT[:, dc, t, :], tp[:])

                # qkT[c, t] for c < 1536: lhsT = w chunk, rhs = xT
                for ct in range(CT):
                    for nh in range(2):
                        ps = ps_qk.tile([128, 512], F32, tag="qk")
                        for kc in range(DC):
                            nc.tensor.matmul(
                                ps[:],
                                wqk[:, kc, ct * 128 : (ct + 1) * 128],
                                xT[:, kc, nh * 4 : (nh + 1) * 4, :],
                                start=(kc == 0),
                                stop=(kc == DC - 1),
                            )
                        dst = qkT[:, ct, nh * 512 : (nh + 1) * 512]
                        if ct < CT // 2:  # q: (psum + b) * 1/sqrt(hd)
                            nc.vector.tensor_scalar(
                                dst, ps[:], bqk_sb[:, ct : ct + 1], SCALE,
                                op0=ADD, op1=MULT,
                            )
                        else:  # k: psum + b
                            nc.vector.tensor_scalar(
                                dst, ps[:], bqk_sb[:, ct : ct + 1], None, op0=ADD
                            )

                # v[t, c]: lhsT = xT chunk, rhs = w_v chunk
                for t in range(NT):
                    ps = ps_v.tile([128, D], F32, tag="v")
                    for kc in range(DC):
                        for j0, j1 in ((0, 512), (512, D)):
                            nc.tensor.matmul(
                                ps[:, j0:j1],
                                xT[:, kc, t, :],
                                wqk[:, kc, 2 * D + j0 : 2 * D + j1],
                                start=(kc == 0),
                                stop=(kc == DC - 1),
                            )
                    psv = ps.rearrange("p (hp two c) -> p hp two c", two=2, c=64)
                    nc.vector.tensor_copy(vpack[:, t, :, 0, 0:64], psv[:, :, 0, :])
                    nc.vector.tensor_copy(vpack[:, t, :, 1, 64:128], psv[:, :, 1, :])

            # ---------------- phase 2: attention per head ----------------
            with (
                tc.tile_pool(name="p2", bufs=2) as p2,
                tc.tile_pool(name="drp", bufs=2, space="DRAM") as drp,
                tc.tile_pool(name="ps_s", bufs=2, space="PSUM") as ps_s,
                tc.tile_pool(name="ps_a", bufs=2, space="PSUM") as ps_a,
            ):
                # w_proj load overlaps attention
                for kc in range(DC):
                    wpst = p2.tile([128, D], F32, tag="wpst")
                    nc.sync.dma_start(
                        wpst[:], wp_d.ap()[kc * 128 : (kc + 1) * 128, :]
                    )
                    nc.any.tensor_copy(wp_sb[:, kc, :], wpst[:])

                for h in range(H):
                    hp, par = divmod(h, 2)
                    off = 64 * par
                    expT = p2.tile([128, KC, N], BF16, tag="expT")
                    pa = ps_a.tile([128, N], F32, tag="pa")
                    for kc in range(KC):
                        ps = ps_s.tile([128, N], F32, tag="s")
                        for nh in range(2):
                            nc.tensor.matmul(
                                ps[:, nh * 512 : (nh + 1) * 512],
                                qkT[off : off + 64, CT // 2 + hp,
                                    kc * 128 : (kc + 1) * 128],
                                qkT[off : off + 64, hp,
                                    nh * 512 : (nh + 1) * 512],
                                start=True,
                                stop=True,
                            )
                        nc.scalar.activation(expT[:, kc, :], ps[:], Exp)
                    for kc in range(KC):
                        for nh in range(2):
                            nc.tensor.matmul(
                                pa[:, nh * 512 : (nh + 1) * 512],
                                vpack[:, kc, hp, par, :],
                                expT[:, kc, nh * 512 : (nh + 1) * 512],
                                start=(kc == 0),
                                stop=(kc == KC - 1),
                            )
                    sumrow = 64 if par == 0 else 0
                    recip = p2.tile([128, N], F32, tag="recip")
                    nc.vector.reciprocal(
                        recip[sumrow : sumrow + 1, :], pa[sumrow : sumrow + 1, :]
                    )
                    # broadcast recip row across 64 partitions via a DRAM
                    # roundtrip (SBUF APs can't have partition step 0, and
                    # gpsimd partition_broadcast doesn't compile on this
                    # walrus; DRAM source APs may broadcast)
                    rdram = drp.tile([1, N], F32, tag="rd")
                    nc.sync.dma_start(rdram[:], recip[sumrow : sumrow + 1, :])
                    bc = p2.tile([128, N], F32, tag="bc")
                    rd_ap = rdram[:]
                    rec_bcast = bass.AP(
                        tensor=rd_ap.tensor, offset=rd_ap.offset,
                        ap=[[0, 64]] + rd_ap.ap[1:],
                    )
                    nc.sync.dma_start(bc[off : off + 64, :], rec_bcast)
                    dst = aT[off : off + 64, hp, :]
                    nc.vector.tensor_tensor(
                        dst, pa[off : off + 64, :], bc[off : off + 64, :], MULT
                    )
                    nc.vector.tensor_scalar_add(
                        dst, dst, bv_sb[off : off + 64, hp : hp + 1]
                    )

            # ---------------- phase 3: proj ----------------
            with (
                tc.tile_pool(name="p3", bufs=2) as p3,
                tc.tile_pool(name="ps_p", bufs=2, space="PSUM") as ps_p,
            ):
                for qt in range(NT):
                    pp = ps_p.tile([128, D], F32, tag="pp")
                    for kc in range(DC):
                        for j0, j1 in ((0, 512), (512, D)):
                            nc.tensor.matmul(
                                pp[:, j0:j1],
                                aT[:, kc, qt * 128 : (qt + 1) * 128],
                                wp_sb[:, kc, j0:j1],
                                start=(kc == 0),
                                stop=(kc == DC - 1),
                            )
                    ob = p3.tile([128, D], F32, tag="ob")
                    nc.vector.tensor_tensor(ob[:], pp[:], bproj_bc[:], ADD)
                    nc.sync.dma_start(
                        out_d.ap()[qt * 128 : (qt + 1) * 128, :], ob[:]
                    )

    if legalize:
        _legalize_waits(nc)
    return nc


_NC_CACHE = {}
LAST_RESULT = None


def kernel(x, w_qkv, b_qkv, w_proj, b_proj):
    global LAST_RESULT
    x = np.ascontiguousarray(np.asarray(x, dtype=np.float32))
    w_qkv = np.ascontiguousarray(np.asarray(w_qkv, dtype=np.float32))
    b_qkv = np.ascontiguousarray(np.asarray(b_qkv, dtype=np.float32))
    w_proj = np.ascontiguousarray(np.asarray(w_proj, dtype=np.float32))
    b_proj = np.ascontiguousarray(np.asarray(b_proj, dtype=np.float32))
    B = x.shape[0]
    assert x.shape == (B, N, D) and B == 8

    if "nc" not in _NC_CACHE:
        _NC_CACHE["nc"] = build()
    nc = _NC_CACHE["nc"]

    in_maps = [
        {"x": x[i], "w_qkv": w_qkv, "b_qkv": b_qkv,
         "w_proj": w_proj, "b_proj": b_proj}
        for i in range(B)
    ]
    trace = bool(int(os.environ.get("KERNEL_TRACE", "0")))
    res = run_bass_kernel_spmd(
        nc, in_maps, core_ids=list(range(8)), trace=trace
    )
    LAST_RESULT = res
    return np.stack([res.results[i]["out"] for i in range(B)], axis=0)
